# revision 2
# baseline (speedup 1.0000x reference)
"""Trainium2 Bass kernel v2 for nn_Attention_72541997629647.

Sharding: segment x head-half. Core c = 2*si + hh owns segment si (768 rows,
the 4 segments are 128-aligned so no boundary masks) and heads
[8*hh, 8*hh+8). Each core computes qkv+RoPE for its 8 heads over its 768
rows, block-diagonal attention (which only needs rows inside the segment),
and a proj partial [DIM, 768] contracted over its 640 attention channels
(5 full 128-partition tiles -> no wasted contraction rows). The host sums
the two partials per segment and adds b_proj.

vs v1 (heads-only sharding): per-core DMA drops 36.7MB -> ~11MB (x slice
instead of full x, bf16 I/O everywhere), proj PE cost drops 25.6us -> 16us,
and psum->sbuf copies move to the idle Pool (gpsimd) engine.

All matmuls run in bf16 (1 cycle/row at any size; fp32 psum accumulate).
The softmax denominator path stays f32/f32r.
"""

import os
import sys

for _p in ("/opt/trn_rl_repo", "/root/.axon_site/_ro/trn_rl_repo"):
    if os.path.isdir(_p) and _p not in sys.path:
        sys.path.insert(0, _p)

import numpy as np

import concourse.bacc as bacc
import concourse.bass as bass
import concourse.mybir as mybir
import concourse.tile as tile
from concourse.bass_utils import run_bass_kernel_spmd
from contextlib import ExitStack

S = 3072
DIM = 1280
H = 16
HD = 80
NCORES = 8
SEG = 768            # rows per segment
HPC = 8              # heads per core
NT = SEG // 128      # 6 s-tiles per core
GROUPS = 4           # head groups of 2 per core
VEXT = 97            # v cols: 80 v + 16 pad + ones at 96

F32 = mybir.dt.float32
F32R = mybir.dt.float32r
BF16 = mybir.dt.bfloat16
NPBF16 = mybir.dt.np(BF16)

CANON_SEGS = tuple((SEG * i, SEG * (i + 1)) for i in range(4))

_CACHE: dict = {}


def _segments_from_cu(cu_seqlens: np.ndarray) -> tuple:
    cu = np.asarray(cu_seqlens).astype(np.int64)
    seg = np.searchsorted(cu, np.arange(S), side="right") - 1
    change = np.nonzero(np.diff(seg))[0]
    starts = np.concatenate([[0], change + 1])
    ends = np.concatenate([change + 1, [S]])
    return tuple((int(a), int(b)) for a, b in zip(starts, ends))


def _chan_runs(h: int):
    """(src_row0, dst_ct, dst_row0, n) runs mapping head h's 80 attention
    channels into the 6x128 packed layout at 96-row pitch (16 pad rows per
    head). Runs are 32+32+16 so every src/dst partition base is 32-aligned
    (hardware requires engine AP partition offsets in {0,32,64,96})."""
    runs = []
    for src, n in ((0, 32), (32, 32), (64, 16)):
        gpos = 96 * h + src
        runs.append((src, gpos // 128, gpos % 128, n))
    return runs


def _build(segments, loop_n: int = 1) -> "bacc.Bacc":
    assert segments == CANON_SEGS
    nc = bacc.Bacc("TRN2", target_bir_lowering=False, debug=False,
                   num_devices=NCORES)

    xblk_d = nc.dram_tensor("xblk", [NT, 128, 10, 128], BF16,
                            kind="ExternalInput")
    wqkvb_d = nc.dram_tensor("wqkvb", [GROUPS, 10, 128, 480], BF16,
                             kind="ExternalInput")
    bqkv_d = nc.dram_tensor("bqkv", [1, GROUPS, 480], BF16,
                            kind="ExternalInput")
    cosb_d = nc.dram_tensor("cosb", [128, NT, HD], F32, kind="ExternalInput")
    sinb_d = nc.dram_tensor("sinb", [128, NT, HD], F32, kind="ExternalInput")
    wptb_d = nc.dram_tensor("wptb", [128, 6, DIM], BF16, kind="ExternalInput")
    ident_d = nc.dram_tensor("ident", [128, 128], BF16, kind="ExternalInput")
    ones_d = nc.dram_tensor("onesrow", [1, 128], BF16, kind="ExternalInput")
    onesf_d = nc.dram_tensor("onesf", [1, 128], F32R, kind="ExternalInput")
    out0_d = nc.dram_tensor("outb0", [2, 128, 5, 512], BF16,
                            kind="ExternalOutput")
    out1a_d = nc.dram_tensor("outb1a", [2, 128, 5, 128], BF16,
                             kind="ExternalOutput")
    out1b_d = nc.dram_tensor("outb1b", [2, 128, 5, 128], BF16,
                             kind="ExternalOutput")

    with tile.TileContext(nc) as tc, ExitStack() as ctx:
        if loop_n > 1:
            ctx.enter_context(tc.For_i(0, loop_n, 1))
        per = ctx.enter_context(tc.tile_pool(name="persist", bufs=1))

        ident_sb = per.tile([128, 128], BF16, tag="ident")
        ones_sb = per.tile([1, 128], BF16, tag="ones")
        onesf_sb = per.tile([1, 128], F32R, tag="onesf")
        bqkv_sb = per.tile([1, GROUPS, 480], BF16, tag="bqkv")
        cos_sb = per.tile([128, NT, HD], F32, tag="cos")
        sin_sb = per.tile([128, NT, HD], F32, tag="sin")
        xt_sb = [per.tile([128, 10, 128], BF16, tag=f"xt{t}", name=f"xt{t}")
                 for t in range(NT)]
        wq_sb = [[per.tile([128, 480], BF16, tag=f"wq{g}_{dp}",
                           name=f"wq{g}_{dp}") for dp in range(10)]
                 for g in range(GROUPS)]
        wp_sb = per.tile([128, 6, DIM], BF16, tag="wp")
        qkTg = [per.tile([HD, 4, SEG], BF16, tag=f"qkT{g}", name=f"qkT{g}")
                for g in range(GROUPS)]
        v_sb = per.tile([128, HPC, NT, VEXT], BF16, tag="v")
        at_sb = per.tile([128, 6, SEG], BF16, tag="at")
        # the 16 pad rows per 96-row head granule are never written by the
        # normalize stage but ARE read by the proj matmul (against zero
        # weights) -- zero the tile so no inf/NaN garbage reaches the PE
        nc.gpsimd.memset(at_sb[:, :, :], 0.0)

        # v pad columns: zeros at 80:96, softmax-denominator ones at 96;
        # memset on the otherwise-idle Pool engine (a broadcast DMA here
        # would emit 6144 tiny descriptors and block the SP queue ~100us)
        nc.gpsimd.memset(v_sb[:, :, :, HD:VEXT - 1], 0.0)
        nc.gpsimd.memset(v_sb[:, :, :, VEXT - 1:VEXT], 1.0)

        # input DMAs: first-compute-first. Issue rate is ~650ns serial per
        # DMA per queue, so spread across SP, ACT, and the Pool software
        # DGE: SP feeds the first group (x tile 0 + wq group 0), ACT takes
        # the small constants and remaining x tiles, Pool streams the other
        # three weight groups in per-chunk DMAs.
        nc.sync.dma_start(out=xt_sb[0], in_=xblk_d[0])
        for dp in range(10):
            nc.sync.dma_start(out=wq_sb[0][dp], in_=wqkvb_d[0, dp])
        nc.sync.dma_start(out=xt_sb[1], in_=xblk_d[1])
        nc.sync.dma_start(out=xt_sb[2], in_=xblk_d[2])
        nc.scalar.dma_start(out=ones_sb, in_=ones_d[:, :])
        nc.scalar.dma_start(out=bqkv_sb, in_=bqkv_d[:, :, :])
        nc.scalar.dma_start(out=cos_sb, in_=cosb_d[:, :, :])
        nc.scalar.dma_start(out=sin_sb, in_=sinb_d[:, :, :])
        nc.scalar.dma_start(out=ident_sb, in_=ident_d[:, :])
        nc.scalar.dma_start(out=onesf_sb, in_=onesf_d[:, :])
        for t in range(3, NT):
            nc.scalar.dma_start(out=xt_sb[t], in_=xblk_d[t])
        for g in range(1, GROUPS):
            for dp in range(10):
                nc.sync.dma_start(out=wq_sb[g][dp], in_=wqkvb_d[g, dp])
        nc.scalar.dma_start(out=wp_sb, in_=wptb_d[:, :, :])

        qpp = ctx.enter_context(tc.tile_pool(name="qpp", bufs=2, space="PSUM"))
        tpp = ctx.enter_context(tc.tile_pool(name="tpp", bufs=1, space="PSUM"))
        scp = ctx.enter_context(tc.tile_pool(name="scp", bufs=3, space="PSUM"))
        app = ctx.enter_context(tc.tile_pool(name="app", bufs=2, space="PSUM"))
        ropep = ctx.enter_context(tc.tile_pool(name="ropet", bufs=4))
        qkrop = ctx.enter_context(tc.tile_pool(name="qkro", bufs=3))
        expp = ctx.enter_context(tc.tile_pool(name="expp", bufs=8))
        smp = ctx.enter_context(tc.tile_pool(name="smalls", bufs=4))
        outp = ctx.enter_context(tc.tile_pool(name="outp", bufs=2))

        def emit_tp(g, t, ro):
            tp = tpp.tile([HD, 4, 128], BF16, tag="tp", name="tpps")
            for j in range(4):
                nc.tensor.transpose(tp[:, j, :], ro[:, HD * j:HD * (j + 1)],
                                    ident_sb)
            nc.vector.tensor_copy(qkTg[g][:, :, 128 * t:128 * (t + 1)], tp)

        def emit_A(g):
            """qkv + rope + v copy + qk transposes for head group g;
            yields once per s-tile so B work of the previous group can be
            interleaved between tiles.

            The transposes for tile t are deferred until after tile t+1's
            qkv matmuls so the PE never waits on the DVE RoPE chain."""
            pending = None
            for t in range(NT):
                qp = qpp.tile([128, 480], F32, tag="qp", name="qkvps")
                for dp in range(5):
                    nc.tensor.matmul(qp[:, :], lhsT=xt_sb[t][:, dp, :],
                                     rhs=wq_sb[g][dp],
                                     start=(dp == 0), stop=False)
                yield
                for dp in range(5, 10):
                    nc.tensor.matmul(qp[:, :], lhsT=xt_sb[t][:, dp, :],
                                     rhs=wq_sb[g][dp],
                                     start=False, stop=False)
                nc.tensor.matmul(qp[:, :], lhsT=ones_sb[:, :],
                                 rhs=bqkv_sb[:, g, :], start=False, stop=True)
                if pending is not None:
                    emit_tp(g, *pending)

                m1 = ropep.tile([128, 320], BF16, tag="m1")
                m2 = ropep.tile([128, 320], BF16, tag="m2")
                qk_h = qp[:, 0:320].rearrange("p (h d) -> p h d", h=4)
                cos_b = cos_sb[:, t:t + 1, :].to_broadcast([128, 4, HD])
                with nc.allow_low_precision("bf16 matmul inputs"):
                    nc.vector.tensor_mul(
                        m1.rearrange("p (h d) -> p h d", h=4), qk_h, cos_b)
                swap = qp[:, 0:320].rearrange(
                    "p (h x d) -> p h x d", h=4, x=2)[:, :, ::-1, :]
                sin_b = sin_sb[:, t:t + 1, :].rearrange(
                    "p t (x d) -> p (t x) d", x=2)[:, None, :, :] \
                    .to_broadcast([128, 4, 2, HD // 2])
                with nc.allow_low_precision("bf16 matmul inputs"):
                    nc.vector.tensor_mul(
                        m2.rearrange("p (h x d) -> p h x d", h=4, x=2),
                        swap, sin_b)
                ro = qkrop.tile([128, 320], BF16, tag="qkro")
                with nc.allow_low_precision("bf16 matmul inputs"):
                    nc.vector.tensor_add(ro, m1, m2)

                with nc.allow_low_precision("bf16 matmul inputs"):
                    nc.scalar.copy(
                        v_sb[:, 2 * g:2 * g + 2, t, 0:HD],
                        qp[:, 320:480].rearrange("p (e d) -> p e d", e=2))
                pending = (t, ro)
                yield
            emit_tp(g, *pending)

        def emit_B(h, qc0, qc1):
            """attention for core-local head h over q columns [qc0, qc1);
            yields between pipeline stages."""
            g, e = h // 2, h % 2
            qT = qkTg[g][:, e]
            kT = qkTg[g][:, 2 + e]
            qna = qc1 - qc0
            ap_ = app.tile([VEXT, 512], F32, tag="ap", name="attps")
            blocks = list(range(NT))
            for g0 in range(0, NT, 4):
                grp = blocks[g0:g0 + 4]
                exs = []
                for j in grp:
                    sc = scp.tile([128, 512], F32, tag="sc", name="scps")
                    nc.tensor.matmul(sc[:, :qna],
                                     lhsT=kT[:, 128 * j:128 * (j + 1)],
                                     rhs=qT[:, qc0:qc1],
                                     start=True, stop=True)
                    ex = expp.tile([128, 512], BF16, tag="expp")
                    nc.scalar.activation(ex[:, :qna], sc[:, :qna],
                                         mybir.ActivationFunctionType.Exp)
                    exs.append(ex)
                yield
                for j, ex in zip(grp, exs):
                    nc.tensor.matmul(ap_[:, :qna], lhsT=v_sb[:, h, j, :],
                                     rhs=ex[:, :qna],
                                     start=(j == 0), stop=(j == NT - 1))
                yield
            den = smp.tile([1, 512], F32R, tag="den", name="den")
            with nc.allow_low_precision("f32r matmul inputs"):
                if qna == 512:
                    nc.scalar.copy(den[:, :qna], ap_[96:97, :qna])
                else:
                    nc.vector.tensor_copy(den[:, :qna], ap_[96:97, :qna])
            yield
            bc = scp.tile([HD, 512], F32, tag="sc", name="bcps")
            nc.tensor.matmul(bc[:, :qna], lhsT=onesf_sb[:, 0:HD],
                             rhs=den[:, :qna], start=True, stop=True)
            rec = smp.tile([HD, 512], F32, tag="rec", name="rec")
            nc.vector.reciprocal(rec[:, :qna], bc[:, :qna])
            for (src, ct, dst, n) in _chan_runs(h):
                with nc.allow_low_precision("bf16 matmul inputs"):
                    nc.vector.tensor_mul(
                        at_sb[dst:dst + n, ct, qc0:qc1],
                        ap_[src:src + n, :qna],
                        rec[src:src + n, :qna])
            yield

        def emit_C(c0, c1, out_d, finer=False):
            """proj for s columns [c0, c1); dense output DMA per half-dim
            chunk (or per m-tile when finer), issue alternating SP/ACT."""
            n = c1 - c0
            for mh in range(2):
                ob = outp.tile([128, 5, n], BF16, tag="outp")
                for mm in range(5):
                    m = 5 * mh + mm
                    pp = qpp.tile([128, 512], F32, tag="qp", name="prps")
                    for ct in range(6):
                        nc.tensor.matmul(
                            pp[:, :n],
                            lhsT=wp_sb[:, ct, 128 * m:128 * (m + 1)],
                            rhs=at_sb[:, ct, c0:c1],
                            start=(ct == 0), stop=(ct == 5))
                    with nc.allow_low_precision("bf16 output"):
                        nc.scalar.copy(ob[:, mm, :], pp[:, :n])
                    if finer:
                        (nc.sync if mm % 2 == 0 else nc.scalar).dma_start(
                            out=out_d[mh, :, mm, :], in_=ob[:, mm, :])
                    yield
                if not finer:
                    (nc.sync if mh == 0 else nc.scalar).dma_start(
                        out=out_d[mh], in_=ob)

        def chain(*gens):
            for gg in gens:
                yield from gg

        def zipgen(gen_a, gen_b):
            """Alternate single steps of two independent streams; drains
            both. Used for head pairs so one head's PE stages fill the
            other's exp-latency holes."""
            a_live, b_live = True, True
            while a_live or b_live:
                if a_live:
                    a_live = next(gen_a, StopIteration) is not StopIteration
                if b_live:
                    b_live = next(gen_b, StopIteration) is not StopIteration
                yield

        def interleave(gen_a, gen_b, ratio):
            """Alternate: one step of gen_a, then `ratio` steps of gen_b.
            Drains both."""
            a_live, b_live = True, True
            while a_live or b_live:
                if a_live:
                    a_live = next(gen_a, StopIteration) is not StopIteration
                if b_live:
                    for _ in range(ratio):
                        if next(gen_b, StopIteration) is StopIteration:
                            b_live = False
                            break

        def gen_group_B(g):
            for e in range(2):
                h = 2 * g + e
                yield from emit_B(h, 0, 512)
                yield from emit_B(h, 512, SEG)

        prev_b = None
        for g in range(GROUPS):
            if prev_b is None:
                for _ in emit_A(g):
                    pass
            else:
                interleave(emit_A(g), prev_b, 2)
            if g < GROUPS - 1:
                prev_b = gen_group_B(g)
        # tail: last group's B with C interleaved once its inputs are ready;
        # the last head's second chunk is split so the final serial
        # norm->proj->DMA chain covers only 128 columns
        h0, h1 = 2 * (GROUPS - 1), 2 * (GROUPS - 1) + 1
        for _ in zipgen(emit_B(h0, 0, 512), emit_B(h1, 0, 512)):
            pass
        interleave(chain(emit_B(h0, 512, SEG), emit_B(h1, 512, 640),
                         emit_B(h1, 640, SEG)),
                   emit_C(0, 512, out0_d), 2)
        for _ in emit_C(512, 640, out1a_d):
            pass
        for _ in emit_C(640, SEG, out1b_d, finer=True):
            pass

    nc.compile()
    return nc


def _prep_inputs(x, cu_seqlens, rotary_pos_emb, w_qkv, b_qkv, w_proj, b_proj):
    """Host-side shard prep. Returns per-core input dicts."""
    scale = np.float32(1.0 / np.sqrt(np.float32(HD)))
    x = np.asarray(x, np.float32)
    w_qkv = np.asarray(w_qkv, np.float32)
    b_qkv = np.asarray(b_qkv, np.float32)
    w_proj = np.asarray(w_proj, np.float32)
    rot = np.asarray(rotary_pos_emb, np.float32)

    cosw = np.concatenate([np.cos(rot), np.cos(rot)], axis=1)
    sinw = np.concatenate([-np.sin(rot), np.sin(rot)], axis=1)

    ident = np.eye(128, dtype=NPBF16)
    onesrow = np.ones((1, 128), dtype=NPBF16)
    onesf = np.ones((1, 128), dtype=np.float32)

    in_maps = []
    for c in range(NCORES):
        si, hh = c // 2, c % 2
        s0 = SEG * si
        heads = list(range(8 * hh, 8 * hh + 8))

        xs = x[s0:s0 + SEG].astype(NPBF16)  # [768, 1280]
        # xblk[t, p, dp, s'] = x[s0+128t+s', 128dp+p]
        xblk = np.ascontiguousarray(
            xs.reshape(NT, 128, 10, 128).transpose(0, 3, 2, 1))

        # w_qkv rows in per-group order [q_a q_b k_a k_b v_a v_b] x 80
        idx = []
        for g in range(GROUPS):
            a, b = heads[2 * g], heads[2 * g + 1]
            for base, hsel in ((0, a), (0, b), (DIM, a), (DIM, b),
                               (2 * DIM, a), (2 * DIM, b)):
                idx.extend(range(base + hsel * HD, base + (hsel + 1) * HD))
        w_c = w_qkv[idx, :].copy()
        b_c = b_qkv[idx].copy()
        for g in range(GROUPS):
            w_c[480 * g:480 * g + 160] *= scale
            b_c[480 * g:480 * g + 160] *= scale
        # wqkvb[g, dp, p, cc] = w_c[480g+cc, 128dp+p] (dense per-dp chunks)
        wqkvb = np.ascontiguousarray(
            w_c.T.reshape(10, 128, GROUPS, 480).transpose(2, 0, 1, 3)
        ).astype(NPBF16)
        bqkvb = np.ascontiguousarray(b_c.reshape(1, GROUPS, 480)).astype(NPBF16)

        # rope tables [128, 6, 80] for this segment's rows
        cosb = np.ascontiguousarray(
            cosw[s0:s0 + SEG].reshape(NT, 128, HD).transpose(1, 0, 2))
        sinb = np.ascontiguousarray(
            sinw[s0:s0 + SEG].reshape(NT, 128, HD).transpose(1, 0, 2))

        # wptb[ct, p, m] = w_proj[m, chan(128ct+p)], chan c -> head
        # heads[c//80], dim c%80
        wptb = np.zeros((6, 128, DIM), np.float32)
        wv = wptb.reshape(768, DIM)
        for hl, habs in enumerate(heads):
            wv[96 * hl:96 * hl + HD] = w_proj[:, habs * HD:(habs + 1) * HD].T
        wptb = np.ascontiguousarray(
            wptb.transpose(1, 0, 2)).astype(NPBF16)  # [128, 6, DIM]

        in_maps.append({
            "xblk": xblk,
            "wqkvb": wqkvb,
            "bqkv": bqkvb,
            "cosb": cosb,
            "sinb": sinb,
            "wptb": np.ascontiguousarray(wptb),
            "ident": ident,
            "onesrow": onesrow,
            "onesf": onesf,
        })
    return in_maps


def run(inputs: dict, trace: bool = False):
    segments = _segments_from_cu(inputs["cu_seqlens"])
    if segments != CANON_SEGS:
        from kernel_legacy import run as legacy_run
        return legacy_run(inputs, trace=trace)
    key = (segments, "v2")
    if key not in _CACHE:
        _CACHE[key] = _build(segments)
    nc = _CACHE[key]
    in_maps = _prep_inputs(
        inputs["x"], inputs["cu_seqlens"], inputs["rotary_pos_emb"],
        inputs["w_qkv"], inputs["b_qkv"], inputs["w_proj"], inputs["b_proj"])
    res = run_bass_kernel_spmd(nc, in_maps, core_ids=list(range(NCORES)),
                               trace=trace)
    acc = np.zeros((DIM, S), np.float64)
    for c, r in enumerate(res.results):
        si = c // 2
        part = np.zeros((128, 10, SEG), np.float64)
        for mh in range(2):
            part[:, 5 * mh:5 * (mh + 1), 0:512] = r["outb0"][mh]
            part[:, 5 * mh:5 * (mh + 1), 512:640] = r["outb1a"][mh]
            part[:, 5 * mh:5 * (mh + 1), 640:768] = r["outb1b"][mh]
        # partial[128m+p, s'] = part[p, m, s']
        acc[:, SEG * si:SEG * (si + 1)] += part.transpose(1, 0, 2).reshape(
            DIM, SEG)
    acc += np.asarray(inputs["b_proj"], np.float64)[:, None]
    out = np.ascontiguousarray(acc.T.astype(np.float32))
    return out, res


def kernel(**inputs) -> np.ndarray:
    out, _ = run(inputs, trace=False)
    return out
